# Initial kernel scaffold
#
"""MoE layer (shared expert + top-2 routed experts) on 8 NeuronCores.

Strategy (expert-parallel, routing-aware):
  - Router (softmax -> top-2 -> renorm) computed on host in float64; it is
    tiny (8192x8) and must match the reference's top-k selection.
  - Core c owns routed expert c: host gathers the tokens routed to expert c
    (~2k of 8192*2 assignments), pads to a common capacity C, and the device
    runs a dense SwiGLU MLP over just those tokens (bf16 matmuls, fp32 accum).
  - The shared expert is data-parallel: core c also runs the shared SwiGLU
    over tokens [c*1024, (c+1)*1024).
  - Combine is done on host: gate-scale each expert's token outputs (exact
    fp32) and scatter-add; every token has exactly two routed contributions.

Device layout: activations are kept transposed ([d, tokens]) so the native
[K, M] weight layouts of ew1/ew2/ew3 feed nc.tensor.matmul directly with no
on-device transposes. All matmul inputs are bf16 (PE full rate + FWL),
accumulation is fp32 in PSUM, outputs returned fp32.
"""

import sys

for _p in ("/opt/trn_rl_repo",):
    if _p not in sys.path:
        sys.path.append(_p)

import numpy as np
import ml_dtypes

import concourse.bass as bass  # noqa: F401  (engine types via nc)
import concourse.mybir as mybir
import concourse.tile as tile
from concourse import bacc
from concourse.bass_utils import run_bass_kernel_spmd

D = 1024
H = 2048
E = 8
N_TOK = 8192  # 4 * 2048
S = N_TOK // E  # shared-expert tokens per core
KD = D // 128  # 8  k-subtiles over d
KH = H // 128  # 16 k-subtiles over h
MH = H // 128  # 16 m-tiles over h
MD = D // 128  # 8  m-tiles over d
NCHUNK = 512

BF = mybir.dt.bfloat16
F32 = mybir.dt.float32

_program_cache: dict[int, "bacc.Bacc"] = {}


def _emit_swiglu(nc, tc, pools, w1_d, w2_d, w3_d, x_d, out_d, T):
    """Emit one SwiGLU MLP: out[d, t] = (silu(w1.T@x) * (w2.T@x)).T @ w3, all
    activations stored [d-part, token-free]. T tokens (multiple of 128)."""
    wpool, xpool, hpool, hspool, opool, ppool, popool = pools

    # Expert weights resident in SBUF for the whole phase (bf16, 12 MiB).
    wa = []
    wb = []
    for k in range(KD):
        ta = wpool.tile([128, H], BF, tag=f"wa{k}")
        nc.sync.dma_start(ta[:], w1_d[:, k, :])
        wa.append(ta)
    for k in range(KD):
        tb = wpool.tile([128, H], BF, tag=f"wb{k}")
        nc.sync.dma_start(tb[:], w2_d[:, k, :])
        wb.append(tb)
    wc = []
    for k in range(KH):
        tc_ = wpool.tile([128, D], BF, tag=f"wc{k}")
        nc.sync.dma_start(tc_[:], w3_d[:, k, :])
        wc.append(tc_)

    n_chunks = (T + NCHUNK - 1) // NCHUNK
    for ni in range(n_chunks):
        n0 = ni * NCHUNK
        nsz = min(NCHUNK, T - n0)

        xc = xpool.tile([128, KD, NCHUNK], BF, tag="xc")
        nc.sync.dma_start(xc[:, :, :nsz], x_d[:, :, n0 : n0 + nsz])

        h = hpool.tile([128, KH, NCHUNK], BF, tag="h")
        for m in range(MH):
            pu = ppool.tile([128, NCHUNK], F32, tag="pu")
            pv = ppool.tile([128, NCHUNK], F32, tag="pv")
            for k in range(KD):
                nc.tensor.matmul(
                    pu[:, :nsz],
                    wa[k][:, m * 128 : (m + 1) * 128],
                    xc[:, k, :nsz],
                    start=(k == 0),
                    stop=(k == KD - 1),
                )
            for k in range(KD):
                nc.tensor.matmul(
                    pv[:, :nsz],
                    wb[k][:, m * 128 : (m + 1) * 128],
                    xc[:, k, :nsz],
                    start=(k == 0),
                    stop=(k == KD - 1),
                )
            hs = hspool.tile([128, NCHUNK], F32, tag="hs")
            nc.scalar.activation(
                hs[:, :nsz], pu[:, :nsz], mybir.ActivationFunctionType.Silu
            )
            nc.vector.tensor_mul(h[:, m, :nsz], hs[:, :nsz], pv[:, :nsz])

        for mo in range(MD):
            po = popool.tile([128, NCHUNK], F32, tag="po")
            for k in range(KH):
                nc.tensor.matmul(
                    po[:, :nsz],
                    wc[k][:, mo * 128 : (mo + 1) * 128],
                    h[:, k, :nsz],
                    start=(k == 0),
                    stop=(k == KH - 1),
                )
            ot = opool.tile([128, NCHUNK], F32, tag="ot")
            nc.vector.tensor_copy(ot[:, :nsz], po[:, :nsz])
            nc.sync.dma_start(out_d[:, mo, n0 : n0 + nsz], ot[:, :nsz])


def _build_program(C: int):
    nc = bacc.Bacc(None, target_bir_lowering=False)

    xg_d = nc.declare_dram_parameter("xg", [128, KD, C], BF, isOutput=False)
    w1_d = nc.declare_dram_parameter("w1", [128, KD, H], BF, isOutput=False)
    w2_d = nc.declare_dram_parameter("w2", [128, KD, H], BF, isOutput=False)
    w3_d = nc.declare_dram_parameter("w3", [128, KH, D], BF, isOutput=False)
    xs_d = nc.declare_dram_parameter("xs", [128, KD, S], BF, isOutput=False)
    s1_d = nc.declare_dram_parameter("s1", [128, KD, H], BF, isOutput=False)
    s2_d = nc.declare_dram_parameter("s2", [128, KD, H], BF, isOutput=False)
    s3_d = nc.declare_dram_parameter("s3", [128, KH, D], BF, isOutput=False)
    outr_d = nc.declare_dram_parameter("out_r", [128, MD, C], F32, isOutput=True)
    outs_d = nc.declare_dram_parameter("out_s", [128, MD, S], F32, isOutput=True)

    with tile.TileContext(nc) as tc:
        with (
            tc.tile_pool(name="wpool", bufs=1) as wpool,
            tc.tile_pool(name="xpool", bufs=3) as xpool,
            tc.tile_pool(name="hpool", bufs=2) as hpool,
            tc.tile_pool(name="hspool", bufs=3) as hspool,
            tc.tile_pool(name="opool", bufs=3) as opool,
            tc.tile_pool(name="ppool", bufs=3, space="PSUM") as ppool,
            tc.tile_pool(name="popool", bufs=2, space="PSUM") as popool,
        ):
            pools = (wpool, xpool, hpool, hspool, opool, ppool, popool)
            _emit_swiglu(nc, tc, pools, w1_d, w2_d, w3_d, xg_d, outr_d, C)
            _emit_swiglu(nc, tc, pools, s1_d, s2_d, s3_d, xs_d, outs_d, S)

    nc.compile()
    return nc


def _get_program(C: int):
    if C not in _program_cache:
        _program_cache[C] = _build_program(C)
    return _program_cache[C]


def _to_dev_layout(a: np.ndarray) -> np.ndarray:
    """[T, d_in] host activation/weight -> [128, d_in//128, T] bf16 device
    layout (d_in on partitions as d = po*128 + pi)."""
    t, din = a.shape
    b = a.T.reshape(din // 128, 128, t).transpose(1, 0, 2)
    return np.ascontiguousarray(b.astype(ml_dtypes.bfloat16))


def _from_dev_layout(a: np.ndarray) -> np.ndarray:
    """[128, dout//128, T] fp32 device output -> [T, dout] host layout."""
    pi, po, t = a.shape
    return a.transpose(1, 0, 2).reshape(pi * po, t).T


def kernel(x, sw1, sw2, sw3, ew1, ew2, ew3, rw, rb):
    xf = np.ascontiguousarray(x.reshape(N_TOK, D), dtype=np.float32)

    # --- host router (float64 to track the fp32 reference's ordering) ---
    logits = xf.astype(np.float64) @ rw.astype(np.float64) + rb.astype(np.float64)
    logits -= logits.max(axis=1, keepdims=True)
    p = np.exp(logits)
    p /= p.sum(axis=1, keepdims=True)
    order = np.argsort(-p, axis=1, kind="stable")
    idx = order[:, :2]  # [N, 2] expert ids, top-2
    w = np.take_along_axis(p, idx, axis=1)
    w = w / w.sum(axis=1, keepdims=True)

    tok_lists = []
    gate_lists = []
    for e in range(E):
        sel = idx == e  # [N, 2]
        any_e = sel.any(axis=1)
        tok = np.nonzero(any_e)[0]
        ge = np.where(sel[tok, 0], w[tok, 0], w[tok, 1])
        tok_lists.append(tok)
        gate_lists.append(ge.astype(np.float64))

    maxT = max(len(t) for t in tok_lists)
    C = max(256, ((maxT + 127) // 128) * 128)

    nc = _get_program(C)

    # --- per-core input maps ---
    w1s = [_to_dev_layout(ew1[e]) for e in range(E)]
    w2s = [_to_dev_layout(ew2[e]) for e in range(E)]
    w3s = [_to_dev_layout(ew3[e]) for e in range(E)]
    s1 = _to_dev_layout(sw1)
    s2 = _to_dev_layout(sw2)
    s3 = _to_dev_layout(sw3)

    in_maps = []
    for e in range(E):
        tok = tok_lists[e]
        xg = np.zeros((C, D), dtype=np.float32)
        xg[: len(tok)] = xf[tok]
        in_maps.append(
            {
                "xg": _to_dev_layout(xg),
                "w1": w1s[e],
                "w2": w2s[e],
                "w3": w3s[e],
                "xs": _to_dev_layout(xf[e * S : (e + 1) * S]),
                "s1": s1,
                "s2": s2,
                "s3": s3,
            }
        )

    res = run_bass_kernel_spmd(nc, in_maps, list(range(E)))

    # --- host combine: shared shards + gated scatter-add of routed outputs ---
    out = np.empty((N_TOK, D), dtype=np.float32)
    for e in range(E):
        out[e * S : (e + 1) * S] = _from_dev_layout(res.results[e]["out_s"])

    all_tok = np.concatenate(tok_lists)
    all_contrib = np.concatenate(
        [
            _from_dev_layout(res.results[e]["out_r"])[: len(tok_lists[e])]
            * gate_lists[e][:, None].astype(np.float32)
            for e in range(E)
        ]
    )
    pos = np.argsort(all_tok, kind="stable")
    # every token has exactly two routed contributions (top-2 routing)
    out += all_contrib[pos[0::2]]
    out += all_contrib[pos[1::2]]

    return out.reshape(x.shape).astype(np.float32)


# revision 1
# speedup vs baseline: 1.1837x; 1.1837x over previous
"""MoE layer (shared expert + top-2 routed experts) on 8 NeuronCores.

Strategy (expert-parallel, routing-aware):
  - Router (softmax -> top-2 -> renorm) computed on host in float64; it is
    tiny (8192x8) and must match the reference's top-k selection.
  - Core c owns routed expert c: host gathers the tokens routed to expert c
    (~2k of 8192*2 assignments), pads to a common capacity C, and the device
    runs a dense SwiGLU MLP over just those tokens (bf16 matmuls, fp32 accum).
  - The shared expert is data-parallel: core c also runs the shared SwiGLU
    over tokens [c*1024, (c+1)*1024).
  - Combine is done on host: gate-scale each expert's token outputs (exact
    fp32) and scatter-add; every token has exactly two routed contributions.

Device layout: activations are kept transposed ([d, tokens]) so the native
[K, M] weight layouts of ew1/ew2/ew3 feed nc.tensor.matmul directly with no
on-device transposes. All matmul inputs are bf16 (PE full rate + FWL),
accumulation is fp32 in PSUM, outputs returned fp32.
"""

import sys

for _p in ("/opt/trn_rl_repo",):
    if _p not in sys.path:
        sys.path.append(_p)

import numpy as np
import ml_dtypes

import concourse.bass as bass  # noqa: F401  (engine types via nc)
import concourse.mybir as mybir
import concourse.tile as tile
from concourse import bacc
from concourse.bass_utils import run_bass_kernel_spmd

D = 1024
H = 2048
E = 8
N_TOK = 8192  # 4 * 2048
S = N_TOK // E  # shared-expert tokens per core
KD = D // 128  # 8  k-subtiles over d
KH = H // 128  # 16 k-subtiles over h
MH = H // 128  # 16 m-tiles over h
MD = D // 128  # 8  m-tiles over d
NCHUNK = 512

BF = mybir.dt.bfloat16
F32 = mybir.dt.float32

_program_cache: dict[int, "bacc.Bacc"] = {}


def _emit_swiglu(nc, tc, pools, w1_d, w2_d, w3_d, x_d, out_d, T):
    """Emit one SwiGLU MLP: out[d, t] = (silu(w1.T@x) * (w2.T@x)).T @ w3, all
    activations stored [d-part, token-free]. T tokens (multiple of 128)."""
    wpool, xpool, hpool, hspool, opool, ppool, popool = pools

    # Expert weights resident in SBUF for the whole phase (bf16, 12 MiB).
    wa = []
    wb = []
    for k in range(KD):
        ta = wpool.tile([128, H], BF, tag=f"wa{k}")
        nc.sync.dma_start(ta[:], w1_d[:, k, :])
        wa.append(ta)
    for k in range(KD):
        tb = wpool.tile([128, H], BF, tag=f"wb{k}")
        nc.sync.dma_start(tb[:], w2_d[:, k, :])
        wb.append(tb)
    wc = []
    for k in range(KH):
        tc_ = wpool.tile([128, D], BF, tag=f"wc{k}")
        nc.sync.dma_start(tc_[:], w3_d[:, k, :])
        wc.append(tc_)

    n_chunks = (T + NCHUNK - 1) // NCHUNK
    for ni in range(n_chunks):
        n0 = ni * NCHUNK
        nsz = min(NCHUNK, T - n0)

        xc = xpool.tile([128, KD, NCHUNK], BF, tag="xc")
        nc.sync.dma_start(xc[:, :, :nsz], x_d[:, :, n0 : n0 + nsz])

        h = hpool.tile([128, KH, NCHUNK], BF, tag="h")
        for m in range(MH):
            pu = ppool.tile([128, NCHUNK], F32, tag="pu")
            pv = ppool.tile([128, NCHUNK], F32, tag="pv")
            for k in range(KD):
                nc.tensor.matmul(
                    pu[:, :nsz],
                    wa[k][:, m * 128 : (m + 1) * 128],
                    xc[:, k, :nsz],
                    start=(k == 0),
                    stop=(k == KD - 1),
                )
            for k in range(KD):
                nc.tensor.matmul(
                    pv[:, :nsz],
                    wb[k][:, m * 128 : (m + 1) * 128],
                    xc[:, k, :nsz],
                    start=(k == 0),
                    stop=(k == KD - 1),
                )
            hs = hspool.tile([128, NCHUNK], F32, tag="hs")
            nc.scalar.activation(
                hs[:, :nsz], pu[:, :nsz], mybir.ActivationFunctionType.Silu
            )
            nc.vector.tensor_mul(h[:, m, :nsz], hs[:, :nsz], pv[:, :nsz])

        for mo in range(MD):
            po = popool.tile([128, NCHUNK], F32, tag="po")
            for k in range(KH):
                nc.tensor.matmul(
                    po[:, :nsz],
                    wc[k][:, mo * 128 : (mo + 1) * 128],
                    h[:, k, :nsz],
                    start=(k == 0),
                    stop=(k == KH - 1),
                )
            ot = opool.tile([128, NCHUNK], F32, tag="ot")
            nc.vector.tensor_copy(ot[:, :nsz], po[:, :nsz])
            nc.sync.dma_start(out_d[:, mo, n0 : n0 + nsz], ot[:, :nsz])


def _build_program(C: int):
    nc = bacc.Bacc(None, target_bir_lowering=False)

    xg_d = nc.declare_dram_parameter("xg", [128, KD, C], BF, isOutput=False)
    w1_d = nc.declare_dram_parameter("w1", [128, KD, H], BF, isOutput=False)
    w2_d = nc.declare_dram_parameter("w2", [128, KD, H], BF, isOutput=False)
    w3_d = nc.declare_dram_parameter("w3", [128, KH, D], BF, isOutput=False)
    xs_d = nc.declare_dram_parameter("xs", [128, KD, S], BF, isOutput=False)
    s1_d = nc.declare_dram_parameter("s1", [128, KD, H], BF, isOutput=False)
    s2_d = nc.declare_dram_parameter("s2", [128, KD, H], BF, isOutput=False)
    s3_d = nc.declare_dram_parameter("s3", [128, KH, D], BF, isOutput=False)
    outr_d = nc.declare_dram_parameter("out_r", [128, MD, C], F32, isOutput=True)
    outs_d = nc.declare_dram_parameter("out_s", [128, MD, S], F32, isOutput=True)

    with tile.TileContext(nc) as tc:
        with (
            tc.tile_pool(name="wpool", bufs=1) as wpool,
            tc.tile_pool(name="xpool", bufs=3) as xpool,
            tc.tile_pool(name="hpool", bufs=2) as hpool,
            tc.tile_pool(name="hspool", bufs=3) as hspool,
            tc.tile_pool(name="opool", bufs=3) as opool,
            tc.tile_pool(name="ppool", bufs=3, space="PSUM") as ppool,
            tc.tile_pool(name="popool", bufs=2, space="PSUM") as popool,
        ):
            pools = (wpool, xpool, hpool, hspool, opool, ppool, popool)
            _emit_swiglu(nc, tc, pools, w1_d, w2_d, w3_d, xg_d, outr_d, C)
            _emit_swiglu(nc, tc, pools, s1_d, s2_d, s3_d, xs_d, outs_d, S)

    nc.compile()
    return nc


def _get_program(C: int):
    if C not in _program_cache:
        _program_cache[C] = _build_program(C)
    return _program_cache[C]


def _to_dev_layout(a: np.ndarray) -> np.ndarray:
    """[T, d_in] host activation/weight -> [128, d_in//128, T] bf16 device
    layout (d_in on partitions as d = po*128 + pi)."""
    t, din = a.shape
    b = a.T.reshape(din // 128, 128, t).transpose(1, 0, 2)
    return np.ascontiguousarray(b.astype(ml_dtypes.bfloat16))


def _from_dev_layout(a: np.ndarray) -> np.ndarray:
    """[128, dout//128, T] fp32 device output -> [T, dout] host layout."""
    pi, po, t = a.shape
    return a.transpose(1, 0, 2).reshape(pi * po, t).T


def kernel(x, sw1, sw2, sw3, ew1, ew2, ew3, rw, rb):
    xf = np.ascontiguousarray(x.reshape(N_TOK, D), dtype=np.float32)

    # --- host router (float64 to track the fp32 reference's ordering) ---
    logits = xf.astype(np.float64) @ rw.astype(np.float64) + rb.astype(np.float64)
    logits -= logits.max(axis=1, keepdims=True)
    p = np.exp(logits)
    p /= p.sum(axis=1, keepdims=True)
    order = np.argsort(-p, axis=1, kind="stable")
    idx = order[:, :2]  # [N, 2] expert ids, top-2
    w = np.take_along_axis(p, idx, axis=1)
    w = w / w.sum(axis=1, keepdims=True)

    tok_lists = []
    gate_lists = []
    for e in range(E):
        sel = idx == e  # [N, 2]
        any_e = sel.any(axis=1)
        tok = np.nonzero(any_e)[0]
        ge = np.where(sel[tok, 0], w[tok, 0], w[tok, 1])
        tok_lists.append(tok)
        gate_lists.append(ge.astype(np.float64))

    maxT = max(len(t) for t in tok_lists)
    C = max(256, ((maxT + 127) // 128) * 128)

    nc = _get_program(C)

    # --- per-core input maps ---
    w1s = [_to_dev_layout(ew1[e]) for e in range(E)]
    w2s = [_to_dev_layout(ew2[e]) for e in range(E)]
    w3s = [_to_dev_layout(ew3[e]) for e in range(E)]
    s1 = _to_dev_layout(sw1)
    s2 = _to_dev_layout(sw2)
    s3 = _to_dev_layout(sw3)

    in_maps = []
    for e in range(E):
        tok = tok_lists[e]
        xg = np.zeros((C, D), dtype=np.float32)
        xg[: len(tok)] = xf[tok]
        in_maps.append(
            {
                "xg": _to_dev_layout(xg),
                "w1": w1s[e],
                "w2": w2s[e],
                "w3": w3s[e],
                "xs": _to_dev_layout(xf[e * S : (e + 1) * S]),
                "s1": s1,
                "s2": s2,
                "s3": s3,
            }
        )

    res = run_bass_kernel_spmd(nc, in_maps, list(range(E)))

    # --- host combine: shared shards + gated scatter-add of routed outputs ---
    out = np.empty((N_TOK, D), dtype=np.float32)
    for e in range(E):
        out[e * S : (e + 1) * S] = _from_dev_layout(res.results[e]["out_s"])

    all_tok = np.concatenate(tok_lists)
    all_contrib = np.concatenate(
        [
            _from_dev_layout(res.results[e]["out_r"])[: len(tok_lists[e])]
            * gate_lists[e][:, None].astype(np.float32)
            for e in range(E)
        ]
    )
    pos = np.argsort(all_tok, kind="stable")
    # every token has exactly two routed contributions (top-2 routing)
    out += all_contrib[pos[0::2]]
    out += all_contrib[pos[1::2]]

    return out.reshape(x.shape).astype(np.float32)
